# revision 10
# baseline (speedup 1.0000x reference)
"""Bidirectional Mamba block on 8 Trainium2 NeuronCores.

Sharding: 8 independent (batch, direction) Mamba branches, one per core.
Per core, the selective scan runs with state channels (d_inner x d_state =
8192) on SBUF partitions (64 tiles of 128) and time on the free axis via the
DVE tensor_tensor_scan instruction. dt/g are replicated x16 across partitions
with tiny PE matmuls against a host-provided 0/1 matrix; the A-multiply is
folded into the per-partition `scale` of the ACT Exp; the d_state reduction
runs as a PE matmul against a 0/1 selection matrix. fwd+bwd partials are
combined with a pair-wise ReduceScatter (cores c and c+4 hold the same
batch); each core then runs LayerNorm+MLP+residual on its half of the
sequence in feature-major layout.
"""

import functools

import numpy as np

B, L, D = 4, 2048, 256
DI, DS, DR, KW = 512, 16, 16, 4
N_CORES = 8
TH = 1024             # per-half sequence length processed on chip
NQ = TH // 512        # 512-wide chunks per half
NCT = DI * DS // 128  # 64 channel tiles (8 d-channels each)
NDT = DI // 128       # 4 d-tiles


def _host_consts():
    # E_sel[:, ctl*128:(ctl+1)*128] replicates rows [8*ctl, 8*ctl+8) of a
    # 128-row tile onto 128 (d_l, s) output partitions.
    E_sel = np.zeros((128, 16 * 128), np.float32)
    F_all = np.zeros((128, 16 * 128), np.float32)
    for dl in range(8):
        for s in range(16):
            for ctl in range(16):
                E_sel[8 * ctl + dl, ctl * 128 + dl * 16 + s] = 1.0
                F_all[dl * 16 + s, ctl * 128 + 8 * ctl + dl] = 1.0
    return E_sel, F_all


@functools.lru_cache(maxsize=1)
def _build():
    import concourse.tile as tile
    from concourse import bacc, mybir

    f32 = mybir.dt.float32
    AF = mybir.ActivationFunctionType
    OP = mybir.AluOpType

    nc = bacc.Bacc("TRN2", target_bir_lowering=False, debug=False,
                   num_devices=N_CORES)

    def din(name, shape):
        return nc.dram_tensor(name, list(shape), f32, kind="ExternalInput")

    d_xT = din("xT", (D, L))
    d_w_inT = din("w_inT", (D, 2 * DI))
    d_conv_w = din("conv_w", (DI, KW))
    d_conv_b = din("conv_b", (DI, 1))
    d_xp_wT = din("xp_wT", (DI, DR + 2 * DS))
    d_dt_wT = din("dt_wT", (DR, DI))
    d_dt_b = din("dt_b", (DI, 1))
    d_alog_p = din("alog_p", (128, NCT))
    d_dp = din("dp", (DI, 1))
    d_w_outT = din("w_outT", (DI, D))
    d_e1 = din("e1", (128, 16 * 128))
    d_f = din("f", (128, 16 * 128))
    d_ln_g = din("ln_g", (D, 1))
    d_ln_b = din("ln_b", (D, 1))
    d_w1T = din("w1T", (D, 2 * D))
    d_b1 = din("b1", (2 * D, 1))
    d_w2T = din("w2T", (2 * D, D))
    d_b2 = din("b2", (D, 1))

    d_out = nc.dram_tensor("out_part", [D, TH], f32, kind="ExternalOutput")

    with tile.TileContext(nc) as tc:
        with (
            tc.tile_pool(name="const", bufs=1) as cp,
            tc.tile_pool(name="dram", bufs=1, space="DRAM") as dpool,
            tc.tile_pool(name="psmm", bufs=4, space="PSUM") as pmm,
            tc.tile_pool(name="psy", bufs=2, space="PSUM") as pyy,
        ):
            cc_in = dpool.tile([2 * D, TH], f32, tag="ccin", name="ccin")
            cc_out = dpool.tile([D, TH], f32, tag="ccout", name="ccout")

            # ---------- constants / weights ----------
            def load_tiles(dram, rows, cols, tag):
                ts = []
                for i in range((rows + 127) // 128):
                    r = min(128, rows - i * 128)
                    t = cp.tile([r, cols], f32, tag=f"{tag}{i}", name=f"{tag}{i}")
                    nc.sync.dma_start(t[:], dram[i * 128:i * 128 + r, :])
                    ts.append(t)
                return ts

            w_in = load_tiles(d_w_inT, D, 2 * DI, "w_in")      # 2 x [128,1024]
            xp_w = load_tiles(d_xp_wT, DI, DR + 2 * DS, "xp")  # 4 x [128,48]
            dt_w = load_tiles(d_dt_wT, DR, DI, "dtw")          # 1 x [16,512]
            w_out = load_tiles(d_w_outT, DI, D, "wo")          # 4 x [128,256]
            w1 = load_tiles(d_w1T, D, 2 * D, "w1")             # 2 x [128,512]
            w2 = load_tiles(d_w2T, 2 * D, D, "w2")             # 4 x [128,256]
            cw = load_tiles(d_conv_w, DI, KW, "cw")            # 4 x [128,4]
            cb = load_tiles(d_conv_b, DI, 1, "cb")
            dtb = load_tiles(d_dt_b, DI, 1, "dtb")
            dpv = load_tiles(d_dp, DI, 1, "dp")
            lng = load_tiles(d_ln_g, D, 1, "lng")
            lnb = load_tiles(d_ln_b, D, 1, "lnb")
            b1v = load_tiles(d_b1, 2 * D, 1, "b1")
            b2v = load_tiles(d_b2, D, 1, "b2")
            e1 = load_tiles(d_e1, 128, 16 * 128, "e1")[0]
            f_sel = load_tiles(d_f, 128, 16 * 128, "fsel")[0]

            # A = -exp(A_log), permuted so column ct is the per-partition
            # scale vector of channel tile ct.
            alog = cp.tile([128, NCT], f32, tag="alog", name="alog")
            nc.sync.dma_start(alog[:], d_alog_p[:, :])
            a_perm = cp.tile([128, NCT], f32, tag="aperm", name="aperm")
            nc.scalar.activation(a_perm[:], alog[:], AF.Exp)
            nc.vector.tensor_scalar_mul(a_perm[:], a_perm[:], -1.0)

            ones_col = cp.tile([128, 1], f32, tag="ones_col", name="ones_col")
            nc.vector.memset(ones_col[:], 1.0)
            ones_row = cp.tile([1, 128], f32, tag="ones_row", name="ones_row")
            nc.vector.memset(ones_row[:], 1.0)

            carry = cp.tile([128, NCT], f32, tag="carry", name="carry")
            eps_t = cp.tile([1, 1], f32, tag="eps", name="eps")
            nc.vector.memset(eps_t[:], 1e-5)

            # ---------- mamba branch, two sequence halves ----------
            with (
                tc.tile_pool(name="seq", bufs=1) as sp,
                tc.tile_pool(name="work", bufs=2) as wp,
            ):
                # persistent: conv left-context lives in the pad columns
                xc_pad = [sp.tile([128, TH + KW - 1], f32, tag=f"xcpad{i}", name=f"xcpad{i}")
                          for i in range(NDT)]
                for e in range(NDT):
                    nc.vector.memset(xc_pad[e][:, 0:KW - 1], 0.0)

                for half in range(2):
                    t0 = half * TH
                    if half == 1:
                        for e in range(NDT):
                            nc.vector.tensor_copy(
                                xc_pad[e][:, 0:KW - 1],
                                xc_pad[e][:, TH:TH + KW - 1])

                    xTh = [wp.tile([128, TH], f32, tag=f"xT{i}", name=f"xT{i}", bufs=1)
                           for i in range(2)]
                    for i in range(2):
                        nc.sync.dma_start(
                            xTh[i][:], d_xT[i * 128:(i + 1) * 128, t0:t0 + TH])

                    zsilu = [sp.tile([128, TH], f32, tag=f"zsilu{i}", name=f"zsilu{i}")
                             for i in range(NDT)]

                    # in_proj: xzT[e,t] = sum_d w_inT[d,e] * xT[d,t]
                    for e in range(8):
                        for q in range(NQ):
                            ps = pmm.tile([128, 512], f32, tag="mm", name="mm")
                            for p in range(2):
                                nc.tensor.matmul(
                                    ps[:], w_in[p][:, e * 128:(e + 1) * 128],
                                    xTh[p][:, q * 512:(q + 1) * 512],
                                    start=(p == 0), stop=(p == 1))
                            if e < NDT:
                                nc.scalar.copy(
                                    xc_pad[e][:, KW - 1 + q * 512:
                                              KW - 1 + (q + 1) * 512], ps[:])
                            else:
                                nc.scalar.activation(
                                    zsilu[e - NDT][:, q * 512:(q + 1) * 512],
                                    ps[:], AF.Silu)

                    # causal depthwise conv + bias + silu
                    xca = [sp.tile([128, TH], f32, tag=f"xca{i}", name=f"xca{i}")
                           for i in range(NDT)]
                    for e in range(NDT):
                        acc = wp.tile([128, TH], f32, tag="convacc", name="convacc", bufs=1)
                        nc.vector.tensor_scalar(
                            acc[:], xc_pad[e][:, 0:TH], cw[e][:, 0:1], None,
                            OP.mult)
                        for k in range(1, KW):
                            nc.vector.scalar_tensor_tensor(
                                acc[:], xc_pad[e][:, k:k + TH],
                                cw[e][:, k:k + 1], acc[:], OP.mult, OP.add)
                        nc.scalar.activation(xca[e][:], acc[:], AF.Silu,
                                             bias=cb[e][:, 0:1])

                    # x_proj -> x_dbl [48, TH]
                    x_dbl = sp.tile([DR + 2 * DS, TH], f32, tag="xdbl", name="xdbl")
                    for q in range(NQ):
                        ps = pmm.tile([DR + 2 * DS, 512], f32, tag="mm", name="mm")
                        for p in range(NDT):
                            nc.tensor.matmul(
                                ps[:], xp_w[p][:],
                                xca[p][:, q * 512:(q + 1) * 512],
                                start=(p == 0), stop=(p == NDT - 1))
                        nc.scalar.copy(x_dbl[:, q * 512:(q + 1) * 512], ps[:])

                    # dt_proj + softplus -> dtT [512, TH]
                    # softplus(x) = ln(exp(x) + 1); Exp and Ln share a table
                    dtT = [sp.tile([128, TH], f32, tag=f"dtT{i}", name=f"dtT{i}")
                           for i in range(NDT)]
                    for e in range(NDT):
                        for q in range(NQ):
                            ps = pmm.tile([128, 512], f32, tag="mm", name="mm")
                            nc.tensor.matmul(
                                ps[:], dt_w[0][:, e * 128:(e + 1) * 128],
                                x_dbl[0:DR, q * 512:(q + 1) * 512])
                            spx = wp.tile([128, 512], f32, tag="spx",
                                          name="spx")
                            nc.scalar.activation(
                                spx[:], ps[:], AF.Exp, bias=dtb[e][:, 0:1])
                            nc.scalar.activation(
                                dtT[e][:, q * 512:(q + 1) * 512], spx[:],
                                AF.Ln, bias=1.0)

                    # B_rep / C_rep: replicate [16, TH] to all 128 partitions
                    b_rep = sp.tile([128, TH], f32, tag="brep", name="brep")
                    c_rep = sp.tile([128, TH], f32, tag="crep", name="crep")
                    for k in range(8):
                        nc.sync.dma_start(b_rep[k * 16:(k + 1) * 16, :],
                                          x_dbl[DR:DR + DS, :])
                        nc.sync.dma_start(c_rep[k * 16:(k + 1) * 16, :],
                                          x_dbl[DR + DS:DR + 2 * DS, :])

                    # g = dt * conv-silu(x); then xca becomes y_D = xca * Dp
                    gT = [sp.tile([128, TH], f32, tag=f"gT{i}", name=f"gT{i}")
                          for i in range(NDT)]
                    for e in range(NDT):
                        nc.vector.tensor_mul(gT[e][:], dtT[e][:], xca[e][:])
                    for e in range(NDT):
                        nc.vector.tensor_scalar(xca[e][:], xca[e][:],
                                                dpv[e][:, 0:1], None, OP.mult)

                    # selective scan over 64 channel tiles
                    ysb = zsilu  # gate output overwrites silu(z) in place
                    yps = {}
                    for ct in range(NCT):
                        dti, ctl_e = divmod(ct, 16)
                        esl = e1[:, ctl_e * 128:(ctl_e + 1) * 128]
                        dA = wp.tile([128, TH], f32, tag="dA", name="dA")
                        dBx = wp.tile([128, TH], f32, tag="dBx", name="dBx")
                        for q in range(NQ):
                            sl = slice(q * 512, (q + 1) * 512)
                            ps = pmm.tile([128, 512], f32, tag="mm", name="mm")
                            nc.tensor.matmul(ps[:], esl, dtT[dti][:, sl])
                            nc.scalar.activation(
                                dA[:, sl], ps[:], AF.Exp,
                                scale=a_perm[:, ct:ct + 1])
                            ps2 = pmm.tile([128, 512], f32, tag="mm", name="mm")
                            nc.tensor.matmul(ps2[:], esl, gT[dti][:, sl])
                            nc.vector.tensor_mul(dBx[:, sl], ps2[:],
                                                 b_rep[:, sl])
                        h = wp.tile([128, TH], f32, tag="h", name="h")
                        init = 0.0 if half == 0 else carry[:, ct:ct + 1]
                        nc.vector.tensor_tensor_scan(
                            h[:], dA[:], dBx[:], init, OP.mult, OP.add)
                        if half == 0:
                            nc.vector.tensor_copy(carry[:, ct:ct + 1],
                                                  h[:, TH - 1:TH])
                        nc.gpsimd.tensor_mul(h[:], h[:], c_rep[:])
                        blk, ctl = divmod(ct, 16)
                        for q in range(NQ):
                            if ctl == 0:
                                yps[q] = pyy.tile([128, 512], f32,
                                                  tag=f"yps{q}",
                                                  name=f"yps{q}")
                            nc.tensor.matmul(
                                yps[q][:],
                                f_sel[:, ctl * 128:(ctl + 1) * 128],
                                h[:, q * 512:(q + 1) * 512],
                                start=(ctl == 0), stop=(ctl == 15))
                        if ctl == 15:
                            for q in range(NQ):
                                sl = slice(q * 512, (q + 1) * 512)
                                tmp = wp.tile([128, 512], f32, tag="ytmp", name="ytmp")
                                nc.vector.tensor_add(tmp[:], yps[q][:],
                                                     xca[blk][:, sl])
                                nc.vector.tensor_mul(zsilu[blk][:, sl], tmp[:],
                                                     zsilu[blk][:, sl])

                    # out_proj -> cc_in rows [half*D : half*D+D]
                    for o in range(2):
                        for q in range(NQ):
                            ps = pmm.tile([128, 512], f32, tag="mm", name="mm")
                            for e in range(NDT):
                                nc.tensor.matmul(
                                    ps[:],
                                    w_out[e][:, o * 128:(o + 1) * 128],
                                    ysb[e][:, q * 512:(q + 1) * 512],
                                    start=(e == 0), stop=(e == NDT - 1))
                            mstg = wp.tile([128, 512], f32, tag="mstg",
                                           name="mstg")
                            nc.scalar.copy(mstg[:], ps[:])
                            nc.sync.dma_start(
                                cc_in[half * D + o * 128:
                                      half * D + (o + 1) * 128,
                                      q * 512:(q + 1) * 512], mstg[:])

            # ---------- combine fwd+bwd across the pair ----------
            nc.gpsimd.collective_compute(
                "ReduceScatter", OP.add,
                replica_groups=[[0, 4], [1, 5], [2, 6], [3, 7]],
                ins=[cc_in.opt()], outs=[cc_out.opt()])

            # ---------- LayerNorm + MLP + residual on [256, TH] ----------
            with tc.tile_pool(name="p2", bufs=1) as p2:
                mh = [p2.tile([128, TH], f32, tag=f"mh{i}", name=f"mh{i}") for i in range(2)]
                for i in range(2):
                    nc.sync.dma_start(mh[i][:],
                                      cc_out[i * 128:(i + 1) * 128, :])

                mu = p2.tile([1, TH], f32, tag="mu", name="mu")
                for q in range(NQ):
                    ps = pmm.tile([1, 512], f32, tag="mm", name="mm")
                    for i in range(2):
                        nc.tensor.matmul(ps[:], ones_col[:],
                                         mh[i][:, q * 512:(q + 1) * 512],
                                         start=(i == 0), stop=(i == 1))
                    nc.scalar.mul(mu[:, q * 512:(q + 1) * 512], ps[:], 1.0 / D)

                mc = [p2.tile([128, TH], f32, tag=f"mc{i}", name=f"mc{i}") for i in range(2)]
                sq = [p2.tile([128, TH], f32, tag=f"sq{i}", name=f"sq{i}") for i in range(2)]
                for q in range(NQ):
                    sl = slice(q * 512, (q + 1) * 512)
                    bps = pmm.tile([128, 512], f32, tag="mm", name="mm")
                    nc.tensor.matmul(bps[:], ones_row[:], mu[:, sl])
                    for i in range(2):
                        nc.vector.tensor_sub(mc[i][:, sl], mh[i][:, sl],
                                             bps[:])
                for i in range(2):
                    nc.scalar.square(sq[i][:], mc[i][:])

                rstd = p2.tile([1, TH], f32, tag="rstd", name="rstd")
                std = p2.tile([1, TH], f32, tag="std", name="std")
                for q in range(NQ):
                    sl = slice(q * 512, (q + 1) * 512)
                    ps = pmm.tile([1, 512], f32, tag="mm", name="mm")
                    for i in range(2):
                        nc.tensor.matmul(ps[:], ones_col[:], sq[i][:, sl],
                                         start=(i == 0), stop=(i == 1))
                    nc.scalar.activation(std[:, sl], ps[:], AF.Sqrt,
                                         scale=1.0 / D, bias=eps_t[:, 0:1])
                nc.vector.reciprocal(rstd[:], std[:])

                hn = [p2.tile([128, TH], f32, tag=f"hn{i}", name=f"hn{i}") for i in range(2)]
                for q in range(NQ):
                    sl = slice(q * 512, (q + 1) * 512)
                    rps = pmm.tile([128, 512], f32, tag="mm", name="mm")
                    nc.tensor.matmul(rps[:], ones_row[:], rstd[:, sl])
                    for i in range(2):
                        nc.vector.tensor_mul(mc[i][:, sl], mc[i][:, sl],
                                             rps[:])
                for i in range(2):
                    nc.scalar.activation(hn[i][:], mc[i][:], AF.Identity,
                                         bias=lnb[i][:, 0:1],
                                         scale=lng[i][:, 0:1])

                h1 = [p2.tile([128, TH], f32, tag=f"h1{i}", name=f"h1{i}") for i in range(4)]
                for o in range(4):
                    for q in range(NQ):
                        ps = pmm.tile([128, 512], f32, tag="mm", name="mm")
                        for i in range(2):
                            nc.tensor.matmul(
                                ps[:], w1[i][:, o * 128:(o + 1) * 128],
                                hn[i][:, q * 512:(q + 1) * 512],
                                start=(i == 0), stop=(i == 1))
                        a1 = p2.tile([128, 512], f32, tag="a1", name="a1")
                        nc.vector.tensor_scalar(a1[:], ps[:], b1v[o][:, 0:1],
                                                None, OP.add)
                        nc.vector.scalar_tensor_tensor(
                            h1[o][:, q * 512:(q + 1) * 512], a1[:], 0.01,
                            a1[:], OP.mult, OP.max)

                outp = [p2.tile([128, TH], f32, tag=f"op{i}", name=f"op{i}")
                        for i in range(2)]
                for o in range(2):
                    for q in range(NQ):
                        sl = slice(q * 512, (q + 1) * 512)
                        ps = pmm.tile([128, 512], f32, tag="mm", name="mm")
                        for e in range(4):
                            nc.tensor.matmul(
                                ps[:], w2[e][:, o * 128:(o + 1) * 128],
                                h1[e][:, sl],
                                start=(e == 0), stop=(e == 3))
                        nc.vector.scalar_tensor_tensor(
                            outp[o][:, sl], ps[:], b2v[o][:, 0:1],
                            mh[o][:, sl], OP.add, OP.add)
                    nc.sync.dma_start(d_out[o * 128:(o + 1) * 128, :],
                                      outp[o][:])

    nc.compile()
    return nc


def _prep_inputs(inputs):
    x = np.asarray(inputs["x"], np.float32)
    E1, F_all = _host_consts()
    in_maps = []
    for c in range(N_CORES):
        b, dr = c % B, c // B
        xv = x[b] if dr == 0 else x[b][::-1]
        alog = np.asarray(inputs["A_log"][dr], np.float32)  # [512, 16]
        alog_p = np.ascontiguousarray(
            alog.reshape(NCT, 8, DS).transpose(1, 2, 0).reshape(128, NCT))
        m = {
            "xT": np.ascontiguousarray(xv.T),
            "w_inT": np.ascontiguousarray(
                np.asarray(inputs["in_proj_w"][dr], np.float32).T),
            "conv_w": np.ascontiguousarray(
                np.asarray(inputs["conv_w"][dr], np.float32)),
            "conv_b": np.ascontiguousarray(
                np.asarray(inputs["conv_b"][dr], np.float32)[:, None]),
            "xp_wT": np.ascontiguousarray(
                np.asarray(inputs["x_proj_w"][dr], np.float32).T),
            "dt_wT": np.ascontiguousarray(
                np.asarray(inputs["dt_proj_w"][dr], np.float32).T),
            "dt_b": np.ascontiguousarray(
                np.asarray(inputs["dt_proj_b"][dr], np.float32)[:, None]),
            "alog_p": alog_p,
            "dp": np.ascontiguousarray(
                np.asarray(inputs["Dp"][dr], np.float32)[:, None]),
            "w_outT": np.ascontiguousarray(
                np.asarray(inputs["out_proj_w"][dr], np.float32).T),
            "e1": E1,
            "f": F_all,
            "ln_g": np.ascontiguousarray(
                np.asarray(inputs["ln_g"], np.float32)[:, None]),
            "ln_b": np.ascontiguousarray(
                np.asarray(inputs["ln_b"], np.float32)[:, None]),
            "w1T": np.ascontiguousarray(
                np.asarray(inputs["w1"], np.float32).T),
            "b1": np.ascontiguousarray(
                np.asarray(inputs["b1"], np.float32)[:, None]),
            "w2T": np.ascontiguousarray(
                np.asarray(inputs["w2"], np.float32).T),
            "b2": np.ascontiguousarray(
                np.asarray(inputs["b2"], np.float32)[:, None]),
        }
        in_maps.append(m)
    return in_maps


def kernel(**inputs):
    from concourse.bass_utils import run_bass_kernel_spmd

    nc = _build()
    in_maps = _prep_inputs(inputs)
    res = run_bass_kernel_spmd(nc, in_maps, list(range(N_CORES)))
    out = np.empty((B, L, D), np.float32)
    for c in range(N_CORES):
        b, dr = c % B, c // B
        out[b, dr * TH:(dr + 1) * TH, :] = res.results[c]["out_part"].T
    return out
